# revision 15
# baseline (speedup 1.0000x reference)
"""Trainium2 Bass kernel for a diagonal-A linear dynamical system (LDS).

    Bu = inputs @ B            [B, T, S]
    h_t = h_{t-1} * A + Bu_t   (scan over T, diagonal A)
    y_t = h_t @ C              [B, T, O]

Shapes: inputs [16, 4096, 256], A [256], B [256, 256], C [256, 256],
h0 [256]; all float32.

Sharding: data-parallel over batch across 8 NeuronCores (2 batches per
core); A/B/C/h0 replicated.

v6 design: mixed-flavor supertiles to balance PE vs DVE.

The DVE TensorTensorScanArith runs ~2cyc/column and only exists on DVE,
so a plain kernel is scan-bound (39us/core). The R=2 pair-step
reformulation halves scan columns but adds 25% PE work:

    R supertile (1024 t):  v_u = u_even @ B' + u_odd @ B  (B'=B diag A)
        scan odd states with A^2;  y_odd = H @ C
        y_even = H_shift @ C' + u_even @ BC   (C'=diag(A) C, BC=B@C)
        -> 40 matmuls n=512, 4 scans of 512 cols
    P supertile (1024 t):  bu = u @ B; scan all states with A; y = h @ C
        -> 32 matmuls n=512, 8 scans of 512 cols

Alternating [R, P, R, P] balances: per R+P pair PE=72 mm (15.5us),
DVE=12 scans (14.7us). All weights host-folded; B,C scaled by 16 so
BC stays fp16-normal (ACT copy multiplies y by 1/256). fp16 data path;
PSUM accumulation and scan state stay fp32.

DMA issue order matters (~0.65us serial issue cost per dma_start on the
Sync engine): W1 and the first u tile (split by i-half) go first.
"""

import numpy as np

import concourse.bacc as bacc
import concourse.bass as bass
import concourse.mybir as mybir
import concourse.tile as tile
from concourse import bass_utils

BATCH, T, D = 16, 4096, 256
NCORES = 8
BLOC = BATCH // NCORES  # batches per core
TT = 1024               # time supertile
NJ = T // TT            # supertiles per sequence
SC = 512                # scan columns / matmul moving width
FLAVORS = ["R", "P", "R", "P"]
R_IDX = {j: r for r, j in enumerate(j for j in range(NJ) if FLAVORS[j] == "R")}
WSCALE = 16.0           # host scales B, C by this; y copy undoes ^2
F32 = mybir.dt.float32
F16 = mybir.dt.float16

_CACHE: dict = {}


def _build_nc():
    nc = bacc.Bacc(trn_type="TRN2", target_bir_lowering=False)

    # R supertiles: u[b,k,p,(j par e)]; P supertiles: u[b,k,p,(j t)]
    u = nc.dram_tensor("u", [BLOC, 2, 128, T], F16, kind="ExternalInput")
    # W1[p, w, k, d]: w=0 Bp=B*diag(A), w=1 B   (i = k*128+p)
    W1d = nc.dram_tensor("W1", [128, 2, 2, D], F16, kind="ExternalInput")
    # W2[p, w, k, d]: w=0 C, w=1 Cp=diag(A)*C, w=2 BC=B@C
    W2d = nc.dram_tensor("W2", [128, 3, 2, D], F16, kind="ExternalInput")
    # S[p, c]: c=0,1 A^2 halves; c=2,3 h0 halves; c=4,5 A halves
    Sd = nc.dram_tensor("S", [128, 6], F32, kind="ExternalInput")
    y = nc.dram_tensor("y", [BLOC, 2, 128, T], F16, kind="ExternalOutput")

    u_rr = u[:].rearrange("b k p (j par e) -> b j p k par e", par=2, e=SC)
    u_rp = u[:].rearrange("b k p (j t) -> b j p k t", t=TT)
    y_rr = y[:].rearrange("b m p (j par e) -> b j p m par e", par=2, e=SC)
    y_rp = y[:].rearrange("b m p (j t) -> b j p m t", t=TT)

    mult = mybir.AluOpType.mult
    add = mybir.AluOpType.add
    INV = 1.0 / (WSCALE * WSCALE)

    with tile.TileContext(nc) as tc:
        with (
            tc.tile_pool(name="const", bufs=1) as const,
            tc.tile_pool(name="upool", bufs=6) as upool,
            tc.tile_pool(name="yrp", bufs=2) as yrp,
            tc.tile_pool(name="ypp", bufs=2) as ypp,
            tc.tile_pool(name="hop", bufs=1) as hop,
            tc.tile_pool(name="hpp", bufs=4) as hpp,
            tc.tile_pool(name="ps_v", bufs=4, space="PSUM") as ps_v,
            tc.tile_pool(name="ps_y", bufs=2, space="PSUM") as ps_y,
        ):
            u_t: dict = {}

            def dma_u(b, j, split=False):
                ut = upool.tile([128, 2, 2, SC], F16, tag="u_t", name="u_t")
                u_t[(b, j)] = ut
                src = u_rr[b, j] if FLAVORS[j] == "R" else u_rp[
                    b, j
                ].rearrange("p k (par e) -> p k par e", par=2)
                if split:  # separate dma per i-half so k=0 lands sooner
                    for k in range(2):
                        nc.sync.dma_start(ut[:, k], src[:, k])
                else:
                    nc.sync.dma_start(ut, src)

            # --- head: W1 then first u tiles, then the rest ---
            W1 = const.tile([128, 2, 2, D], F16, name="W1")
            nc.sync.dma_start(W1, W1d[:])
            dma_u(0, 0, split=True)
            dma_u(1, 0)
            W2 = const.tile([128, 3, 2, D], F16, name="W2")
            nc.sync.dma_start(W2, W2d[:])
            Sc = const.tile([128, 6], F32, name="Sc")
            nc.sync.dma_start(Sc, Sd[:])

            Bp_sb, B_sb = W1[:, 0], W1[:, 1]          # [128, 2, D]
            C_sb, Cp_sb, BC_sb = W2[:, 0], W2[:, 1], W2[:, 2]
            A2_col, h0c, A_col = Sc[:, 0:2], Sc[:, 2:4], Sc[:, 4:6]

            ones = const.tile([128, SC], F32, name="ones")
            nc.vector.memset(ones, 1.0)
            A2_bc = const.tile([128, 2, SC], F32, name="A2_bc")
            A_bc = const.tile([128, 2, SC], F32, name="A_bc")
            for m in range(2):
                nc.scalar.mul(A2_bc[:, m], ones, mul=A2_col[:, m : m + 1])
                nc.scalar.mul(A_bc[:, m], ones, mul=A_col[:, m : m + 1])

            # R-supertile odd states: [seg][0]=guard h_{t0-1}, then 512 odds
            hO = hop.tile([128, BLOC, 2, len(R_IDX), 1 + SC], F16, name="hO")
            hP: dict = {}  # plain supertile full states, per (b, j)

            def prev_state(b, m, j):
                """AP of h_{j*TT - 1} (an odd state)."""
                if j == 0:
                    return h0c[:, m : m + 1]
                if FLAVORS[j - 1] == "R":
                    r = R_IDX[j - 1]
                    return hO[:, b, m, r, SC : SC + 1]
                return hP[(b, j - 1)][:, m, TT - 1 : TT]

            def emit_R(b, j):
                r = R_IDX[j]
                ut = u_t[(b, j)]
                # guard col for the H_shift matmul
                for m in range(2):
                    nc.scalar.copy(hO[:, b, m, r, 0:1], prev_state(b, m, j))
                vs = []
                for m in range(2):
                    ms = slice(m * 128, (m + 1) * 128)
                    v = ps_v.tile([128, SC], F32, tag="v", name="v")
                    vs.append(v)
                    for k in range(2):
                        nc.tensor.matmul(
                            v, Bp_sb[:, k, ms], ut[:, k, 0],
                            start=(k == 0), stop=False,
                        )
                        nc.tensor.matmul(
                            v, B_sb[:, k, ms], ut[:, k, 1],
                            start=False, stop=(k == 1),
                        )
                for m in range(2):
                    nc.vector.tensor_tensor_scan(
                        hO[:, b, m, r, 1 : 1 + SC],
                        A2_bc[:, m],
                        vs[m],
                        prev_state(b, m, j),
                        op0=mult,
                        op1=add,
                    )

            def emit_y_R(b, j):
                r = R_IDX[j]
                ysb = yrp.tile([128, 2, 2, SC], F16, tag="ysbr", name="ysbr")
                for m in range(2):
                    ms = slice(m * 128, (m + 1) * 128)
                    yod = ps_y.tile([128, SC], F32, tag="yod", name="yod")
                    yev = ps_y.tile([128, SC], F32, tag="yev", name="yev")
                    for k in range(2):
                        nc.tensor.matmul(
                            yod, C_sb[:, k, ms], hO[:, b, k, r, 1 : 1 + SC],
                            start=(k == 0), stop=(k == 1),
                        )
                    for k in range(2):
                        nc.tensor.matmul(
                            yev, Cp_sb[:, k, ms], hO[:, b, k, r, 0:SC],
                            start=(k == 0), stop=False,
                        )
                    for k in range(2):
                        nc.tensor.matmul(
                            yev, BC_sb[:, k, ms], u_t[(b, j)][:, k, 0],
                            start=False, stop=(k == 1),
                        )
                    nc.scalar.mul(ysb[:, m, 1, :], yod, mul=INV)
                    nc.scalar.mul(ysb[:, m, 0, :], yev, mul=INV)
                nc.sync.dma_start(y_rr[b, j], ysb)

            def emit_P_chunk(b, j, c):
                ut = u_t[(b, j)]  # [128, k, par(=t half), SC] natural t
                if c == 0:
                    hP[(b, j)] = hpp.tile([128, 2, TT], F16, tag="hp", name="hp")
                hp = hP[(b, j)]
                vs = []
                for m in range(2):
                    ms = slice(m * 128, (m + 1) * 128)
                    v = ps_v.tile([128, SC], F32, tag="v", name="v")
                    vs.append(v)
                    for k in range(2):
                        nc.tensor.matmul(
                            v, B_sb[:, k, ms], ut[:, k, c],
                            start=(k == 0), stop=(k == 1),
                        )
                for m in range(2):
                    init = (
                        prev_state(b, m, j)
                        if c == 0
                        else hp[:, m, c * SC - 1 : c * SC]
                    )
                    nc.vector.tensor_tensor_scan(
                        hp[:, m, c * SC : (c + 1) * SC],
                        A_bc[:, m],
                        vs[m],
                        init,
                        op0=mult,
                        op1=add,
                    )

            def emit_y_P(b, j):
                hp = hP[(b, j)]
                ysb = ypp.tile([128, 2, TT], F16, tag="ysbp", name="ysbp")
                for c in range(2):
                    for m in range(2):
                        ms = slice(m * 128, (m + 1) * 128)
                        yp = ps_y.tile(
                            [128, SC], F32, tag=("yod" if m == 0 else "yev"),
                            name="yp",
                        )
                        for k in range(2):
                            nc.tensor.matmul(
                                yp, C_sb[:, k, ms],
                                hp[:, k, c * SC : (c + 1) * SC],
                                start=(k == 0), stop=(k == 1),
                            )
                        nc.scalar.mul(
                            ysb[:, m, c * SC : (c + 1) * SC], yp, mul=INV
                        )
                nc.sync.dma_start(y_rp[b, j], ysb)

            def emit_y_flavor(b, j):
                (emit_y_R if FLAVORS[j] == "R" else emit_y_P)(b, j)

            for j in range(NJ):
                if j + 1 < NJ:
                    for b in range(BLOC):
                        dma_u(b, j + 1)
                if FLAVORS[j] == "R":
                    for b in range(BLOC):
                        emit_R(b, j)
                else:
                    for c in range(2):
                        for b in range(BLOC):
                            emit_P_chunk(b, j, c)
                if j >= 1:
                    for b in range(BLOC):
                        emit_y_flavor(b, j - 1)
            for b in range(BLOC):
                emit_y_flavor(b, NJ - 1)

    nc.compile()
    return nc


def _get_nc():
    if "nc" not in _CACHE:
        _CACHE["nc"] = _build_nc()
    return _CACHE["nc"]


def make_in_maps(inputs, A, B, C, h0):
    u = np.asarray(inputs, dtype=np.float32)
    # [B, T, 2, 128] -> [B, 2, 128, T]; even/odd split on R supertiles only
    uT = u.reshape(BATCH, T, 2, 128).transpose(0, 2, 3, 1)
    u5 = uT.reshape(BATCH, 2, 128, NJ, SC, 2).copy()
    for j in range(NJ):
        if FLAVORS[j] == "R":
            u5[:, :, :, j] = u5[:, :, :, j].transpose(0, 1, 2, 4, 3).reshape(
                BATCH, 2, 128, SC, 2
            )
    uT = np.ascontiguousarray(u5).reshape(BATCH, 2, 128, T).astype(np.float16)

    Af = np.asarray(A, np.float32)
    Bf = np.asarray(B, np.float32) * WSCALE
    Cf = np.asarray(C, np.float32) * WSCALE
    Bp = Bf * Af[None, :]          # B * diag(A)
    Cp = Cf * Af[:, None]          # diag(A) * C
    BC = Bf @ Cf                   # (16B) @ (16C) = 256 * B@C

    def wsplit(M):  # [256, 256] -> [128, 2, 256] (p, k, d)
        return M.reshape(2, 128, D).transpose(1, 0, 2)

    W1 = np.ascontiguousarray(
        np.stack([wsplit(Bp), wsplit(Bf)], axis=1)
    ).astype(np.float16)
    W2 = np.ascontiguousarray(
        np.stack([wsplit(Cf), wsplit(Cp), wsplit(BC)], axis=1)
    ).astype(np.float16)
    A2 = (Af * Af).reshape(2, 128).T
    h02 = (np.asarray(h0, np.float32) * WSCALE).reshape(2, 128).T
    A1 = Af.reshape(2, 128).T
    S = np.ascontiguousarray(
        np.concatenate([A2, h02, A1], axis=1), dtype=np.float32
    )
    core_consts = {"W1": W1, "W2": W2, "S": S}
    return [
        {"u": np.ascontiguousarray(uT[c * BLOC : (c + 1) * BLOC]), **core_consts}
        for c in range(NCORES)
    ]


def kernel(inputs, A, B, C, h0, _trace=False):
    nc = _get_nc()
    in_maps = make_in_maps(inputs, A, B, C, h0)
    res = bass_utils.run_bass_kernel_spmd(
        nc, in_maps, core_ids=list(range(NCORES)), trace=_trace
    )
    outs = []
    for r in res.results:
        yT = r["y"].astype(np.float32)  # [BLOC, 2, 128, T]
        y5 = yT.reshape(BLOC, 2, 128, NJ, 2, SC).copy()
        for j in range(NJ):
            if FLAVORS[j] == "R":  # [par, e] -> natural [e, par]
                y5[:, :, :, j] = y5[:, :, :, j].transpose(0, 1, 2, 4, 3).reshape(
                    BLOC, 2, 128, 2, SC
                )
            else:  # already natural: [2, SC] halves ARE time order
                pass
        yT = y5.reshape(BLOC, 2, 128, T)
        outs.append(np.moveaxis(yT, 3, 1).reshape(BLOC, T, D))
    out = np.concatenate(outs, axis=0)
    if _trace:
        _CACHE["last_result"] = res
    return out
